# revision 17
# baseline (speedup 1.0000x reference)
"""GCNII kernel v4 for 8 Trainium2 NeuronCores.

Per layer: per-edge source rows are fetched with dma_gather (fp16, 256B
descs, 4 queues) from the AllGathered node table; scatter one-hot tiles
are built ON-CHIP on the Vector engine (iota==dl) from resident per-piece
dl columns (layer-invariant); the per-edge weight factorizes as
norm = (c*dinv[src]) * (c*dinv[dst]): the src factor is pre-scaled into
the table rows, the dst factor is a per-column multiply in the epilogue,
so the one-hot is pure 0/1 and self-loops are ordinary slots.

Cells (dst-block x src-quad) are packed UNALIGNED at their max-over-cores
size after a host-side node-balancing permutation (within each quarter),
so descriptor count ~= edge count.  Chunks that straddle cell boundaries
issue one matmul per overlapped block ("pieces") with a masked dl column.
Region-pipelined AllGather (4 sub-shards per layer).
"""
import os
import numpy as np

import concourse.bass as bass
import concourse.bacc as bacc
import concourse.mybir as mybir
import concourse.tile as tile
from concourse.bass_utils import run_bass_kernel_spmd

f32 = mybir.dt.float32
f16 = mybir.dt.float16
i16 = mybir.dt.int16
i32 = mybir.dt.int32

N = 100000
E = 1000000
IN_DIM = 256
HID = 128
LAYERS = 8
ALPHA = 0.1
THETA = 0.5
NCORES = 8
SHARD = N // NCORES          # 12500
B = 128
NBLK = -(-SHARD // B)        # 98
NQ = 4
GRP = 4
NGRP = -(-NBLK // GRP)       # 25
RSZ = [3200, 3200, 3200, 2900]           # rows per core per region
RBASE = [0, 3200, 6400, 9600]
TRSZ = [r * NCORES for r in RSZ]         # table region rows
AG_GROUP = {}
for _r, _last in ((0, 24), (1, 49), (2, 74), (3, 97)):
    AG_GROUP[_last // GRP] = _r


def _reg_of(j):
    return np.minimum(j // 3200, 3)


def _wrap_idx(a):
    s = a.reshape(-1, 16).T.astype(np.int16)
    return np.tile(s, (8, 1))


def _balance(d4):
    """Mean-targeted balance within each (core, quarter); returns node_at
    (new position -> node id)."""
    node_at = np.empty(N, np.int64)
    for c in range(NCORES):
        for qt in range(4):
            lo = c * SHARD + RBASE[qt]
            n = RSZ[qt]
            nodes = np.arange(lo, lo + n)
            nb = -(-n // B)
            caps = np.full(nb, B, np.int64)
            caps[-1] = n - (nb - 1) * B
            dv = d4[nodes].astype(np.float64)
            capsf = (caps / B)[:, None]
            target = dv.sum(0)[None, :] / nb * capsf
            order_n = np.argsort(-dv.max(1), kind="stable")
            sums = np.zeros((nb, NQ))
            fill = np.zeros(nb, np.int64)
            bins_of = np.zeros(n, np.int64)
            scale = np.sqrt(np.maximum(target, 1.0))
            for i in order_n:
                dev = ((sums + dv[i]) - target) / scale
                score = dev.max(1) + 1e9 * (fill >= caps)
                j = int(np.argmin(score))
                sums[j] += dv[i]
                bins_of[i] = j
                fill[j] += 1
            node_at[lo:lo + n] = nodes[np.argsort(bins_of, kind="stable")]
    return node_at


def _build_layout(v):
    """v[b, q] = cell capacity (max count over cores).

    Returns calls: list of dicts (g, q, scol, nidx, nch, pbase, pieces,
    off), plus total slot columns (chunk-padded) and total piece count.
    """
    calls = []
    scol = 0
    pcol = 0
    for g in range(NGRP):
        blo, bhi = g * GRP, min((g + 1) * GRP, NBLK)
        for q in range(NQ):
            sizes = [int(v[b, q]) for b in range(blo, bhi)]
            off = np.concatenate([[0], np.cumsum(sizes)]).astype(np.int64)
            S = int(off[-1])
            nidx = -(-S // 16) * 16
            nch = -(-nidx // 128) if nidx else 0
            pieces = []
            for cch in range(nch):
                w0, w1 = cch * 128, cch * 128 + 128
                for bi, b in enumerate(range(blo, bhi)):
                    if off[bi] < w1 and off[bi + 1] > w0 and sizes[bi] > 0:
                        pieces.append((cch, b))
            calls.append(dict(g=g, q=q, scol=scol, nidx=nidx, nch=nch,
                              pbase=pcol, pieces=pieces, off=off))
            scol += nch * 128
            pcol += len(pieces)
    return calls, scol, pcol


def _build_program(calls, total_slots, total_pieces, maxcall, maxpieces,
                   nchb_of):
    nc = bacc.Bacc("TRN2", target_bir_lowering=False, debug=False,
                   num_devices=NCORES, num_swdge_queues=NQ)

    t_xT = nc.dram_tensor("xT", [IN_DIM, SHARD], f16, kind="ExternalInput")
    t_idx = nc.dram_tensor("gidx", [128, total_slots // 16], i16,
                           kind="ExternalInput")
    t_dl = nc.dram_tensor("dl16", [128, total_pieces], f16,
                          kind="ExternalInput")
    t_wrow = nc.dram_tensor("wrow", [128, SHARD], f16, kind="ExternalInput")
    t_iota = nc.dram_tensor("iota", [128, 128], f16, kind="ExternalInput")
    t_dinv = nc.dram_tensor("dinvrow", [128, NBLK], f32, kind="ExternalInput")
    NW = 14
    t_wp = nc.dram_tensor("wpack", [NW * 128, HID], f16, kind="ExternalInput")
    t_bias = nc.dram_tensor("bias", [128, 4], f32, kind="ExternalInput")

    t_out = nc.dram_tensor("out_s", [128, SHARD], f32, kind="ExternalOutput")
    t_xv = nc.dram_tensor("xv_s", [128, SHARD], f32, kind="ExternalOutput")
    t_xt = nc.dram_tensor("xt_s", [128, SHARD], f32, kind="ExternalOutput")

    t_tbl = [[nc.dram_tensor(f"tbl{i}_{r}", [TRSZ[r], HID], f16,
                             kind="Internal", addr_space="Shared")
              for r in range(4)] for i in range(2)]
    t_agin = [[nc.dram_tensor(f"agin{i}_{r}", [RSZ[r], HID], f16,
                              kind="Internal") for r in range(4)]
              for i in range(2)]

    RG = [list(range(NCORES))]

    with tile.TileContext(nc) as tc:
        with (
            tc.tile_pool(name="persist", bufs=1) as pp,
            tc.tile_pool(name="gp", bufs=10) as gp,
            tc.tile_pool(name="ohs", bufs=6) as ohs,
            tc.tile_pool(name="ep", bufs=8) as ep,
            tc.tile_pool(name="psa", bufs=6, space="PSUM") as psa,
            tc.tile_pool(name="pse", bufs=2, space="PSUM") as pse,
        ):
            s_idx = pp.tile([128, total_slots // 16], i16)
            nc.sync.dma_start(out=s_idx[:], in_=t_idx.ap())
            s_dl = pp.tile([128, total_pieces], f16)
            nc.sync.dma_start(out=s_dl[:], in_=t_dl.ap())
            s_wrow = pp.tile([128, SHARD], f16)
            nc.sync.dma_start(out=s_wrow[:], in_=t_wrow.ap())
            s_iota = pp.tile([128, 128], f16)
            nc.sync.dma_start(out=s_iota[:], in_=t_iota.ap())
            s_dinv = pp.tile([128, NBLK], f32)
            nc.sync.dma_start(out=s_dinv[:], in_=t_dinv.ap())
            sw = []
            for k in range(NW):
                w = pp.tile([128, HID], f16, tag=f"w{k}")
                nc.sync.dma_start(out=w[:], in_=t_wp.ap()[k * 128:(k + 1) * 128, :])
                sw.append(w)
            (w1a, w1b) = sw[0:2]
            su = sw[2:10]
            w2, wv, wt, b1row = sw[10:14]
            s_bias = pp.tile([128, 4], f32)
            nc.sync.dma_start(out=s_bias[:], in_=t_bias.ap())
            h0T = pp.tile([128, SHARD], f16, tag="h0T")
            h8T = pp.tile([128, SHARD], f16, tag="h8T")

            # zero gather bufs once: trailing non-gathered lanes of partial
            # chunks must hold finite values for the 0-weighted matmul.
            for _ in range(10):
                gz = gp.tile([128, maxcall * 128], f16, tag="gt", name="gt")
                nc.vector.memset(gz[:], 0.0)

            def nb_of(b):
                return min(B, SHARD - b * B)

            # ---------------- W1 phase (feature-major h) ----------------
            for b in range(NBLK):
                nb = nb_of(b)
                cols = slice(b * B, b * B + nb)
                xa = ep.tile([128, B], f16, tag="xa")
                xb = ep.tile([128, B], f16, tag="xb")
                nc.sync.dma_start(out=xa[:, :nb], in_=t_xT.ap()[0:128, cols])
                nc.sync.dma_start(out=xb[:, :nb], in_=t_xT.ap()[128:256, cols])
                ps = psa.tile([128, B], f32, tag="agg", name="agg")
                nc.tensor.matmul(ps[:, :nb], lhsT=w1a[:], rhs=xa[:, :nb],
                                 start=True, stop=False)
                nc.tensor.matmul(ps[:, :nb], lhsT=w1b[:], rhs=xb[:, :nb],
                                 start=False, stop=True)
                hT = ep.tile([128, B], f16, tag="hT")
                nc.vector.tensor_scalar(out=hT[:, :nb], in0=ps[:, :nb],
                                        scalar1=s_bias[:, 0:1], scalar2=None,
                                        op0=mybir.AluOpType.add)
                nc.scalar.activation(out=h0T[:, cols], in_=hT[:, :nb],
                                     func=mybir.ActivationFunctionType.Relu,
                                     scale=ALPHA)
                ps_n = pse.tile([128, B], f32, tag="pp")
                nc.tensor.matmul(ps_n[:nb, :], lhsT=xa[:, :nb], rhs=w1a[:],
                                 start=True, stop=False)
                nc.tensor.matmul(ps_n[:nb, :], lhsT=xb[:, :nb], rhs=w1b[:],
                                 start=False, stop=True)
                hrow = ep.tile([128, HID], f16, tag="hrow")
                nc.vector.tensor_tensor(out=hrow[:nb, :], in0=ps_n[:nb, :],
                                        in1=b1row[:nb, :],
                                        op=mybir.AluOpType.add)
                # table rows pre-scaled: t = relu(h * c*dinv)  (dinv > 0)
                nc.scalar.activation(out=hrow[:nb, :], in_=hrow[:nb, :],
                                     func=mybir.ActivationFunctionType.Relu,
                                     scale=s_dinv[:nb, b:b + 1])
                r = min(b // 25, 3)
                rows = slice(b * B - RBASE[r], b * B - RBASE[r] + nb)
                nc.sync.dma_start(out=t_agin[0][r].ap()[rows, :],
                                  in_=hrow[:nb, :])
                if b in (24, 49, 74, 97):
                    nc.gpsimd.collective_compute(
                        "AllGather", mybir.AluOpType.bypass,
                        replica_groups=RG,
                        ins=[t_agin[0][b // 25].ap()],
                        outs=[t_tbl[0][b // 25].ap()])

            # ---------------- conv layers ----------------
            for l in range(LAYERS):
                tbl_cur = t_tbl[l % 2]
                tbl_nxt = t_tbl[(l + 1) % 2]
                agin_nxt = t_agin[(l + 1) % 2]
                for g in range(NGRP):
                    blo, bhi = g * GRP, min((g + 1) * GRP, NBLK)
                    aggs = [psa.tile([128, B], f32, tag="agg", name="agg")
                            for _ in range(bhi - blo)]
                    gcalls = [cl for cl in calls
                              if cl["g"] == g and cl["nch"] > 0]
                    gts = {}
                    for cl in gcalls:
                        q = cl["q"]
                        gt = gp.tile([128, maxcall * 128], f16, tag="gt",
                                     name="gt")
                        nc.gpsimd.dma_gather(
                            out_ap=gt[:, :cl["nch"] * 128].rearrange(
                                "p (c f) -> p c f", f=HID),
                            in_ap=tbl_cur[q].ap(),
                            idxs_ap=s_idx[:, cl["scol"] // 16:
                                          cl["scol"] // 16 + cl["nidx"] // 16],
                            num_idxs=cl["nidx"],
                            num_idxs_reg=cl["nidx"],
                            elem_size=HID,
                            single_packet=False,
                            queue_num=q,
                        )
                        npc = len(cl["pieces"])
                        oh = ohs.tile([128, maxpieces * 128], f16, tag="oh",
                                      name="oh")
                        nc.vector.tensor_tensor(
                            out=oh[:, :npc * 128].rearrange(
                                "p (k f) -> p k f", f=128),
                            in0=s_iota[:].unsqueeze(1).broadcast_to(
                                [128, npc, 128]),
                            in1=s_dl[:, cl["pbase"]:cl["pbase"] + npc
                                     ].unsqueeze(2).broadcast_to(
                                [128, npc, 128]),
                            op=mybir.AluOpType.is_equal)
                        gts[q] = (gt, oh, cl)
                    done = {b: 0 for b in range(blo, bhi)}
                    for q in range(NQ):
                        if q not in gts:
                            continue
                        gt, oh, cl = gts[q]
                        for pi, (cch, b) in enumerate(cl["pieces"]):
                            ps_t = aggs[b - blo]
                            k = done[b]
                            nc.tensor.matmul(
                                ps_t[:],
                                lhsT=gt[:, cch * 128:(cch + 1) * 128],
                                rhs=oh[:, pi * 128:(pi + 1) * 128],
                                start=(k == 0), stop=(k == nchb_of[b] - 1))
                            done[b] = k + 1
                    # epilogue (feature-major agg)
                    for b in range(blo, bhi):
                        nb = nb_of(b)
                        cols = slice(b * B, b * B + nb)
                        ps_t = aggs[b - blo]
                        t0 = ep.tile([128, B], f16, tag="t0")
                        nc.vector.tensor_tensor(
                            out=t0[:, :nb], in0=ps_t[:, :nb],
                            in1=s_wrow[:, cols],
                            op=mybir.AluOpType.mult)
                        aggT = ep.tile([128, B], f16, tag="aggT")
                        nc.vector.tensor_tensor(
                            out=aggT[:, :nb], in0=t0[:, :nb],
                            in1=h0T[:, cols],
                            op=mybir.AluOpType.add)
                        if l < LAYERS - 1:
                            ps2 = pse.tile([128, B], f32, tag="pp")
                            nc.tensor.matmul(ps2[:nb, :], lhsT=aggT[:, :nb],
                                             rhs=su[l][:], start=True,
                                             stop=True)
                            hn = ep.tile([128, HID], f16, tag="hn")
                            nc.scalar.activation(
                                out=hn[:nb, :], in_=ps2[:nb, :],
                                func=mybir.ActivationFunctionType.Relu,
                                scale=s_dinv[:nb, b:b + 1])
                            r = min(b // 25, 3)
                            rows = slice(b * B - RBASE[r],
                                         b * B - RBASE[r] + nb)
                            nc.sync.dma_start(out=agin_nxt[r].ap()[rows, :],
                                              in_=hn[:nb, :])
                        else:
                            ps2 = pse.tile([128, B], f32, tag="pp")
                            nc.tensor.matmul(ps2[:, :nb], lhsT=su[l][:],
                                             rhs=aggT[:, :nb],
                                             start=True, stop=True)
                            nc.scalar.activation(
                                out=h8T[:, cols], in_=ps2[:, :nb],
                                func=mybir.ActivationFunctionType.Relu)
                            psh = psa.tile([128, B], f32, tag="agg",
                                           name="agg")
                            nc.tensor.matmul(psh[:, :nb], lhsT=w2[:],
                                             rhs=h8T[:, cols],
                                             start=True, stop=True)
                            ob32 = ep.tile([128, B], f32, tag="ob32")
                            nc.vector.tensor_scalar(
                                out=ob32[:, :nb], in0=psh[:, :nb],
                                scalar1=s_bias[:, 1:2], scalar2=None,
                                op0=mybir.AluOpType.add)
                            nc.sync.dma_start(out=t_out.ap()[:, cols],
                                              in_=ob32[:, :nb])
                            ob16 = ep.tile([128, B], f16, tag="ob16")
                            nc.scalar.activation(
                                out=ob16[:, :nb], in_=ob32[:, :nb],
                                func=mybir.ActivationFunctionType.Copy)
                            for wmat, bcol, tdst, tg in (
                                    (wv, 2, t_xv, "xv"), (wt, 3, t_xt, "xt")):
                                ps3 = psa.tile([128, B], f32, tag="agg",
                                               name="agg")
                                nc.tensor.matmul(ps3[:, :nb], lhsT=wmat[:],
                                                 rhs=ob16[:, :nb],
                                                 start=True, stop=True)
                                vb = ep.tile([128, B], f32, tag=tg)
                                nc.vector.tensor_scalar(
                                    out=vb[:, :nb], in0=ps3[:, :nb],
                                    scalar1=s_bias[:, bcol:bcol + 1],
                                    scalar2=None, op0=mybir.AluOpType.add)
                                nc.scalar.activation(
                                    out=vb[:, :nb], in_=vb[:, :nb],
                                    func=mybir.ActivationFunctionType.Relu)
                                nc.sync.dma_start(out=tdst.ap()[:, cols],
                                                  in_=vb[:, :nb])
                    if l < LAYERS - 1 and g in AG_GROUP:
                        r = AG_GROUP[g]
                        nc.gpsimd.collective_compute(
                            "AllGather", mybir.AluOpType.bypass,
                            replica_groups=RG,
                            ins=[agin_nxt[r].ap()],
                            outs=[tbl_nxt[r].ap()])

    nc.compile()
    return nc


def _install_profile_hook():
    """Dev-only: register the axon NTFF profiling hook (KERNEL_TRACE=1)."""
    import sys
    import types
    if "antenv.axon_hooks" in sys.modules:
        return
    try:
        mod = types.ModuleType("antenv.axon_hooks")
        state = {"hook": None}
        mod.set_axon_ntff_profile_hook = lambda h: state.__setitem__("hook", h)
        mod.get_axon_ntff_profile_hook = lambda: state["hook"]
        sys.modules["antenv.axon_hooks"] = mod
        import antenv
        antenv.axon_hooks = mod
        sys.path.insert(0, "/root/.axon_site")
        from trn_agent_boot.trn_boot import _ntff_profile_via_ctypes
        mod.set_axon_ntff_profile_hook(
            _ntff_profile_via_ctypes("/opt/axon/libaxon_pjrt.so"))
    except Exception as e:  # profiling is best-effort
        print("profile hook install failed:", e)


def kernel(**inputs):
    x = np.asarray(inputs["x"], dtype=np.float32)
    ei = np.asarray(inputs["edge_index"]).astype(np.int64)
    W1 = np.asarray(inputs["W1"], np.float32)
    b1 = np.asarray(inputs["b1"], np.float32)
    conv_w = np.asarray(inputs["conv_w"], np.float32)
    W2 = np.asarray(inputs["W2"], np.float32)
    b2 = np.asarray(inputs["b2"], np.float32)
    Wv = np.asarray(inputs["Wv"], np.float32)
    bv = np.asarray(inputs["bv"], np.float32)
    Wt = np.asarray(inputs["Wt"], np.float32)
    bt = np.asarray(inputs["bt"], np.float32)

    src = ei[0]
    dst = ei[1]

    # ---- node balancing permutation (within each core-quarter slice) ----
    d4 = np.zeros((N, NQ), np.int32)
    np.add.at(d4, (dst, _reg_of(src % SHARD)), 1)
    d4[np.arange(N), _reg_of(np.arange(N) % SHARD)] += 1
    node_at = _balance(d4)
    inv_perm = np.empty(N, np.int64)
    inv_perm[node_at] = np.arange(N)
    x = x[node_at]
    src = inv_perm[src]
    dst = inv_perm[dst]

    deg = np.bincount(dst, minlength=N).astype(np.float64) + 1.0
    dinv = 1.0 / np.sqrt(deg)
    croot = np.sqrt(1.0 - ALPHA)

    a_src = np.concatenate([src, np.arange(N, dtype=np.int64)])
    a_dst = np.concatenate([dst, np.arange(N, dtype=np.int64)])

    core = a_dst // SHARD
    blk = (a_dst % SHARD) // B
    s_c = a_src // SHARD
    s_j = a_src % SHARD
    quad = _reg_of(s_j)
    rsz_a = np.array(RSZ)[quad]
    rb_a = np.array(RBASE)[quad]
    g_idx = s_c * rsz_a + (s_j - rb_a)           # idx within table region

    cell = (core * NBLK + blk) * NQ + quad
    ncell = NCORES * NBLK * NQ
    cnt = np.bincount(cell, minlength=ncell).reshape(NCORES, NBLK, NQ)
    v = cnt.max(axis=0)

    calls, total_slots, total_pieces = _build_layout(v)
    maxcall = max(cl["nch"] for cl in calls)
    maxpieces = max(len(cl["pieces"]) for cl in calls)
    nchb_of = np.zeros(NBLK, np.int64)
    for cl in calls:
        for (cch, b) in cl["pieces"]:
            nchb_of[b] += 1

    order = np.argsort(cell, kind="stable")
    cs = cell[order]
    counts = np.bincount(cell, minlength=ncell)
    pos0 = np.concatenate([[0], np.cumsum(counts)[:-1]])
    rank = np.arange(len(cs)) - pos0[cs]
    b_o = (cs // NQ) % NBLK
    q_o = cs % NQ
    c_o = cs // (NQ * NBLK)
    cell_start = np.zeros((NBLK, NQ), np.int64)
    for cl in calls:
        blo = cl["g"] * GRP
        for bi, b in enumerate(range(blo, min(blo + GRP, NBLK))):
            cell_start[b, cl["q"]] = cl["scol"] + cl["off"][bi]
    slot = cell_start[b_o, q_o] + rank

    idx_arr = np.zeros((NCORES, total_slots), np.int32)
    dl_arr = np.full((NCORES, total_slots), -1.0, np.float32)
    idx_arr[c_o, slot] = g_idx[order]
    dl_arr[c_o, slot] = (a_dst[order] % SHARD - b_o * B).astype(np.float32)

    # piece dl columns [NCORES, 128, total_pieces], masked to each cell
    dlp = np.full((NCORES, 128, total_pieces), -1.0, np.float32)
    for cl in calls:
        blo = cl["g"] * GRP
        for pi, (cch, b) in enumerate(cl["pieces"]):
            bi = b - blo
            lo, hi = int(cl["off"][bi]), int(cl["off"][bi + 1])
            p0 = cch * 128
            s0, s1 = max(lo, p0), min(hi, p0 + 128)
            lanes = np.arange(s0 - p0, s1 - p0)
            slots_r = cl["scol"] + np.arange(s0, s1)
            dlp[:, lanes, cl["pbase"] + pi] = dl_arr[:, slots_r]
    print(f"kernel_v4: slots={total_slots} pieces={total_pieces} "
          f"calls/layer={len(calls)} maxcall={maxcall} "
          f"maxpieces={maxpieces} descs={sum(c['nidx'] for c in calls)}")

    # weights pack
    wpack = np.zeros((14 * 128, HID), np.float32)
    wpack[0:128] = W1[0:128]
    wpack[128:256] = W1[128:256]
    for l in range(LAYERS):
        beta = float(np.log(THETA / (l + 1) + 1.0))
        wpack[(2 + l) * 128:(3 + l) * 128] = \
            (1.0 - beta) * np.eye(HID, dtype=np.float32) + beta * conv_w[l]
    wpack[10 * 128:11 * 128] = W2
    wpack[11 * 128:12 * 128] = Wv
    wpack[12 * 128:13 * 128] = Wt
    wpack[13 * 128:14 * 128] = np.tile(b1, (128, 1))
    wpack16 = wpack.astype(np.float16)
    biaspk = np.stack([b1, b2, bv, bt], axis=1).astype(np.float32)
    iota16 = np.tile(np.arange(128, dtype=np.float16), (128, 1))

    in_maps = []
    for c in range(NCORES):
        dv = np.zeros((128, NBLK), np.float32)
        rows = np.arange(SHARD) + c * SHARD
        dvflat = (croot * dinv[rows]).astype(np.float32)
        dv[:, :SHARD // B] = dvflat[:(SHARD // B) * B].reshape(-1, B).T
        tail = SHARD - (SHARD // B) * B
        if tail:
            dv[:tail, NBLK - 1] = dvflat[(SHARD // B) * B:]
        wrow = np.tile(dvflat.astype(np.float16), (128, 1))
        in_maps.append({
            "xT": np.ascontiguousarray(
                x[c * SHARD:(c + 1) * SHARD].T).astype(np.float16),
            "gidx": _wrap_idx(idx_arr[c]),
            "dl16": np.ascontiguousarray(dlp[c].astype(np.float16)),
            "wrow": np.ascontiguousarray(wrow),
            "iota": iota16,
            "dinvrow": dv,
            "wpack": wpack16,
            "bias": biaspk,
        })

    nc = _build_program(calls, total_slots, total_pieces, maxcall, maxpieces,
                        nchb_of)

    trace = os.environ.get("KERNEL_TRACE", "0") == "1"
    if trace:
        _install_profile_hook()
    res = run_bass_kernel_spmd(nc, in_maps, core_ids=list(range(NCORES)),
                               trace=trace)
    if trace:
        kernel.last_res = res

    def unshard(key):
        full = np.concatenate([np.asarray(res.results[c][key]).T
                               for c in range(NCORES)], axis=0)
        outp = np.empty_like(full)
        outp[node_at] = full
        return outp

    return (unshard("out_s"), unshard("xv_s"), unshard("xt_s"))


# revision 26
# speedup vs baseline: 1.1076x; 1.1076x over previous
"""GCNII kernel v4 for 8 Trainium2 NeuronCores.

Per layer: per-edge source rows are fetched with dma_gather (fp16, 256B
descs, 4 queues) from the AllGathered node table; scatter one-hot tiles
are built ON-CHIP on the Vector engine (iota==dl) from resident per-piece
dl columns (layer-invariant); the per-edge weight factorizes as
norm = (c*dinv[src]) * (c*dinv[dst]): the src factor is pre-scaled into
the table rows, the dst factor is a per-column multiply in the epilogue,
so the one-hot is pure 0/1 and self-loops are ordinary slots.

Cells (dst-block x src-quad) are packed UNALIGNED at their max-over-cores
size after a host-side node-balancing permutation (within each quarter),
so descriptor count ~= edge count.  Chunks that straddle cell boundaries
issue one matmul per overlapped block ("pieces") with a masked dl column.
Region-pipelined AllGather (4 sub-shards per layer).
"""
import os
import numpy as np

import concourse.bass as bass
import concourse.bacc as bacc
import concourse.mybir as mybir
import concourse.tile as tile
from concourse.bass_utils import run_bass_kernel_spmd

f32 = mybir.dt.float32
f16 = mybir.dt.float16
i16 = mybir.dt.int16
i32 = mybir.dt.int32
f8 = mybir.dt.float8e4

N = 100000
E = 1000000
IN_DIM = 256
HID = 128
LAYERS = 8
ALPHA = 0.1
THETA = 0.5
NCORES = 8
SHARD = N // NCORES          # 12500
B = 128
NBLK = -(-SHARD // B)        # 98
NQ = 4
GRP = 4
NGRP = -(-NBLK // GRP)       # 25
RSZ = [3200, 3200, 3200, 2900]           # rows per core per region
RBASE = [0, 3200, 6400, 9600]
TRSZ = [r * NCORES for r in RSZ]         # table region rows
AG_GROUP = {}
for _r, _last in ((0, 24), (1, 49), (2, 74), (3, 97)):
    AG_GROUP[_last // GRP] = _r


def _reg_of(j):
    return np.minimum(j // 3200, 3)


def _wrap_idx(a):
    s = a.reshape(-1, 16).T.astype(np.int16)
    return np.tile(s, (8, 1))


def _balance(d4):
    """Mean-targeted balance within each (core, quarter); returns node_at
    (new position -> node id)."""
    node_at = np.empty(N, np.int64)
    for c in range(NCORES):
        for qt in range(4):
            lo = c * SHARD + RBASE[qt]
            n = RSZ[qt]
            nodes = np.arange(lo, lo + n)
            nb = -(-n // B)
            caps = np.full(nb, B, np.int64)
            caps[-1] = n - (nb - 1) * B
            dv = d4[nodes].astype(np.float64)
            capsf = (caps / B)[:, None]
            target = dv.sum(0)[None, :] / nb * capsf
            order_n = np.argsort(-dv.max(1), kind="stable")
            sums = np.zeros((nb, NQ))
            fill = np.zeros(nb, np.int64)
            bins_of = np.zeros(n, np.int64)
            scale = np.sqrt(np.maximum(target, 1.0))
            for i in order_n:
                dev = ((sums + dv[i]) - target) / scale
                score = dev.max(1) + 1e9 * (fill >= caps)
                j = int(np.argmin(score))
                sums[j] += dv[i]
                bins_of[i] = j
                fill[j] += 1
            node_at[lo:lo + n] = nodes[np.argsort(bins_of, kind="stable")]
    return node_at


def _build_layout(v):
    """v[b, q] = cell capacity (max count over cores).

    Returns calls: list of dicts (g, q, scol, nidx, nch, pbase, pieces,
    off), plus total slot columns (chunk-padded) and total piece count.
    """
    calls = []
    scol = 0
    pcol = 0
    for g in range(NGRP):
        blo, bhi = g * GRP, min((g + 1) * GRP, NBLK)
        for q in range(NQ):
            sizes = [int(v[b, q]) for b in range(blo, bhi)]
            off = np.concatenate([[0], np.cumsum(sizes)]).astype(np.int64)
            S = int(off[-1])
            nidx = -(-S // 16) * 16
            nch = -(-nidx // 128) if nidx else 0
            pieces = []
            for cch in range(nch):
                w0, w1 = cch * 128, cch * 128 + 128
                for bi, b in enumerate(range(blo, bhi)):
                    if off[bi] < w1 and off[bi + 1] > w0 and sizes[bi] > 0:
                        pieces.append((cch, b))
            calls.append(dict(g=g, q=q, scol=scol, nidx=nidx, nch=nch,
                              pbase=pcol, pieces=pieces, off=off))
            scol += nch * 128
            pcol += len(pieces)
    return calls, scol, pcol


def _build_program(calls, total_slots, total_pieces, maxcall, maxpieces,
                   nchb_of):
    nc = bacc.Bacc("TRN2", target_bir_lowering=False, debug=False,
                   num_devices=NCORES, num_swdge_queues=NQ)

    t_xT = nc.dram_tensor("xT", [IN_DIM, SHARD], f16, kind="ExternalInput")
    t_idx = nc.dram_tensor("gidx", [128, total_slots // 16], i16,
                           kind="ExternalInput")
    t_dl = nc.dram_tensor("dl16", [128, total_pieces], f16,
                          kind="ExternalInput")
    t_wrow = nc.dram_tensor("wrow", [128, SHARD], f16, kind="ExternalInput")
    t_iota = nc.dram_tensor("iota", [128, 128], f16, kind="ExternalInput")
    t_ident = nc.dram_tensor("ident", [128, 128], f16, kind="ExternalInput")
    t_dinv = nc.dram_tensor("dinvrow", [128, NBLK], f32, kind="ExternalInput")
    NW = 14
    t_wp = nc.dram_tensor("wpack", [NW * 128, HID], f16, kind="ExternalInput")
    t_bias = nc.dram_tensor("bias", [128, 4], f32, kind="ExternalInput")

    t_out = nc.dram_tensor("out_s", [128, SHARD], f32, kind="ExternalOutput")
    t_xv = nc.dram_tensor("xv_s", [128, SHARD], f32, kind="ExternalOutput")
    t_xt = nc.dram_tensor("xt_s", [128, SHARD], f32, kind="ExternalOutput")

    t_tbl = [[nc.dram_tensor(f"tbl{i}_{r}", [TRSZ[r], HID], f16,
                             kind="Internal", addr_space="Shared")
              for r in range(4)] for i in range(2)]
    t_agin = [[nc.dram_tensor(f"agin{i}_{r}", [RSZ[r], HID], f16,
                              kind="Internal") for r in range(4)]
              for i in range(2)]

    RG = [list(range(NCORES))]

    with tile.TileContext(nc) as tc:
        with (
            tc.tile_pool(name="persist", bufs=1) as pp,
            tc.tile_pool(name="gp", bufs=8) as gp,
            tc.tile_pool(name="ohs", bufs=8) as ohs,
            tc.tile_pool(name="ep", bufs=8) as ep,
            tc.tile_pool(name="psa", bufs=6, space="PSUM") as psa,
            tc.tile_pool(name="pse", bufs=2, space="PSUM") as pse,
        ):
            s_idx = pp.tile([128, total_slots // 16], i16)
            nc.sync.dma_start(out=s_idx[:], in_=t_idx.ap())
            s_dl = pp.tile([128, total_pieces], f16)
            nc.sync.dma_start(out=s_dl[:], in_=t_dl.ap())
            s_wrow = pp.tile([128, SHARD], f16)
            nc.sync.dma_start(out=s_wrow[:], in_=t_wrow.ap())
            s_iota = pp.tile([128, 128], f16)
            nc.sync.dma_start(out=s_iota[:], in_=t_iota.ap())
            s_ident = pp.tile([128, 128], f16)
            nc.sync.dma_start(out=s_ident[:], in_=t_ident.ap())
            t_loc = pp.tile([128, NBLK * HID], f16, tag="tloc")
            s_dinv = pp.tile([128, NBLK], f32)
            nc.sync.dma_start(out=s_dinv[:], in_=t_dinv.ap())
            sw = []
            for k in range(NW):
                w = pp.tile([128, HID], f16, tag=f"w{k}")
                nc.sync.dma_start(out=w[:], in_=t_wp.ap()[k * 128:(k + 1) * 128, :])
                sw.append(w)
            (w1a, w1b) = sw[0:2]
            su = sw[2:10]
            w2, wv, wt, b1row = sw[10:14]
            s_bias = pp.tile([128, 4], f32)
            nc.sync.dma_start(out=s_bias[:], in_=t_bias.ap())
            h0T = pp.tile([128, SHARD], f16, tag="h0T")
            h8T = pp.tile([128, SHARD], f16, tag="h8T")

            # zero gather bufs once: trailing non-gathered lanes of partial
            # chunks must hold finite values for the 0-weighted matmul.
            for _ in range(8):
                gz = gp.tile([128, maxcall * 128], f16, tag="gt", name="gt")
                nc.vector.memset(gz[:], 0.0)

            def nb_of(b):
                return min(B, SHARD - b * B)

            # ---------------- W1 phase (feature-major h) ----------------
            for b in range(NBLK):
                nb = nb_of(b)
                cols = slice(b * B, b * B + nb)
                xa = ep.tile([128, B], f16, tag="xa")
                xb = ep.tile([128, B], f16, tag="xb")
                nc.sync.dma_start(out=xa[:, :nb], in_=t_xT.ap()[0:128, cols])
                nc.sync.dma_start(out=xb[:, :nb], in_=t_xT.ap()[128:256, cols])
                ps = psa.tile([128, B], f32, tag="agg", name="agg")
                nc.tensor.matmul(ps[:, :nb], lhsT=w1a[:], rhs=xa[:, :nb],
                                 start=True, stop=False)
                nc.tensor.matmul(ps[:, :nb], lhsT=w1b[:], rhs=xb[:, :nb],
                                 start=False, stop=True)
                hT = ep.tile([128, B], f16, tag="hT")
                nc.vector.tensor_scalar(out=hT[:, :nb], in0=ps[:, :nb],
                                        scalar1=s_bias[:, 0:1], scalar2=None,
                                        op0=mybir.AluOpType.add)
                nc.scalar.activation(out=h0T[:, cols], in_=hT[:, :nb],
                                     func=mybir.ActivationFunctionType.Relu,
                                     scale=ALPHA)
                ps_n = pse.tile([128, B], f32, tag="pp")
                nc.tensor.matmul(ps_n[:nb, :], lhsT=xa[:, :nb], rhs=w1a[:],
                                 start=True, stop=False)
                nc.tensor.matmul(ps_n[:nb, :], lhsT=xb[:, :nb], rhs=w1b[:],
                                 start=False, stop=True)
                hrow = ep.tile([128, HID], f16, tag="hrow")
                nc.vector.tensor_tensor(out=hrow[:nb, :], in0=ps_n[:nb, :],
                                        in1=b1row[:nb, :],
                                        op=mybir.AluOpType.add)
                # table rows pre-scaled: t = relu(h * c*dinv)  (dinv > 0)
                nc.scalar.activation(out=t_loc[:nb, b * HID:b * HID + HID],
                                     in_=hrow[:nb, :],
                                     func=mybir.ActivationFunctionType.Relu,
                                     scale=s_dinv[:nb, b:b + 1])
                r = min(b // 25, 3)
                rows = slice(b * B - RBASE[r], b * B - RBASE[r] + nb)
                nc.sync.dma_start(out=t_agin[0][r].ap()[rows, :],
                                  in_=t_loc[:nb, b * HID:b * HID + HID])
                if b in (24, 49, 74, 97):
                    nc.gpsimd.collective_compute(
                        "AllGather", mybir.AluOpType.bypass,
                        replica_groups=RG,
                        ins=[t_agin[0][b // 25].ap()],
                        outs=[t_tbl[0][b // 25].ap()])

            # ---------------- conv layers ----------------
            for l in range(LAYERS):
                tbl_cur = t_tbl[l % 2]
                tbl_nxt = t_tbl[(l + 1) % 2]
                agin_nxt = t_agin[(l + 1) % 2]
                for g in range(NGRP):
                    blo, bhi = g * GRP, min((g + 1) * GRP, NBLK)
                    aggs = [psa.tile([128, B], f32, tag="agg", name="agg")
                            for _ in range(bhi - blo)]
                    gcalls = [cl for cl in calls
                              if cl["g"] == g and cl["nch"] > 0]
                    gts = {}
                    for cl in gcalls:
                        q = cl["q"]
                        gt = gp.tile([128, maxcall * 128], f16, tag="gt",
                                     name="gt")
                        nc.gpsimd.dma_gather(
                            out_ap=gt[:, :cl["nch"] * 128].rearrange(
                                "p (c f) -> p c f", f=HID),
                            in_ap=tbl_cur[q].ap(),
                            idxs_ap=s_idx[:, cl["scol"] // 16:
                                          cl["scol"] // 16 + cl["nidx"] // 16],
                            num_idxs=cl["nidx"],
                            num_idxs_reg=cl["nidx"],
                            elem_size=HID,
                            single_packet=False,
                            queue_num=q,
                        )
                        npc = len(cl["pieces"])
                        oh = ohs.tile([128, maxpieces * 128], f8, tag="oh",
                                      name="oh")
                        nc.vector.tensor_tensor(
                            out=oh[:, :npc * 128].rearrange(
                                "p (k f) -> p k f", f=128),
                            in0=s_iota[:].unsqueeze(1).broadcast_to(
                                [128, npc, 128]),
                            in1=s_dl[:, cl["pbase"]:cl["pbase"] + npc
                                     ].unsqueeze(2).broadcast_to(
                                [128, npc, 128]),
                            op=mybir.AluOpType.is_equal)
                        gts[q] = (gt, oh, cl)
                    done = {}
                    for b in range(blo, bhi):
                        nb = nb_of(b)
                        nc.tensor.matmul(
                            aggs[b - blo][:, :nb],
                            lhsT=t_loc[:nb, b * HID:b * HID + HID],
                            rhs=s_ident[:nb, :nb],
                            start=True, stop=(nchb_of[b] == 1))
                        done[b] = 1
                    for q in range(NQ):
                        if q not in gts:
                            continue
                        gt, oh, cl = gts[q]
                        for pi, (cch, b) in enumerate(cl["pieces"]):
                            ps_t = aggs[b - blo]
                            k = done[b]
                            nc.tensor.matmul(
                                ps_t[:],
                                lhsT=gt[:, cch * 128:(cch + 1) * 128],
                                rhs=oh[:, pi * 128:(pi + 1) * 128],
                                start=(k == 0), stop=(k == nchb_of[b] - 1))
                            done[b] = k + 1
                    # epilogue (feature-major agg)
                    for b in range(blo, bhi):
                        nb = nb_of(b)
                        cols = slice(b * B, b * B + nb)
                        ps_t = aggs[b - blo]
                        t0 = ep.tile([128, B], f16, tag="t0")
                        nc.vector.tensor_tensor(
                            out=t0[:, :nb], in0=ps_t[:, :nb],
                            in1=s_wrow[:, cols],
                            op=mybir.AluOpType.mult)
                        aggT = ep.tile([128, B], f16, tag="aggT")
                        nc.vector.tensor_tensor(
                            out=aggT[:, :nb], in0=t0[:, :nb],
                            in1=h0T[:, cols],
                            op=mybir.AluOpType.add)
                        if l < LAYERS - 1:
                            ps2 = pse.tile([128, B], f32, tag="pp")
                            nc.tensor.matmul(ps2[:nb, :], lhsT=aggT[:, :nb],
                                             rhs=su[l][:], start=True,
                                             stop=True)
                            nc.scalar.activation(
                                out=t_loc[:nb, b * HID:b * HID + HID],
                                in_=ps2[:nb, :],
                                func=mybir.ActivationFunctionType.Relu,
                                scale=s_dinv[:nb, b:b + 1])
                            r = min(b // 25, 3)
                            rows = slice(b * B - RBASE[r],
                                         b * B - RBASE[r] + nb)
                            nc.sync.dma_start(out=agin_nxt[r].ap()[rows, :],
                                              in_=t_loc[:nb,
                                                        b * HID:b * HID + HID])
                        else:
                            ps2 = pse.tile([128, B], f32, tag="pp")
                            nc.tensor.matmul(ps2[:, :nb], lhsT=su[l][:],
                                             rhs=aggT[:, :nb],
                                             start=True, stop=True)
                            nc.scalar.activation(
                                out=h8T[:, cols], in_=ps2[:, :nb],
                                func=mybir.ActivationFunctionType.Relu)
                            psh = psa.tile([128, B], f32, tag="agg",
                                           name="agg")
                            nc.tensor.matmul(psh[:, :nb], lhsT=w2[:],
                                             rhs=h8T[:, cols],
                                             start=True, stop=True)
                            ob32 = ep.tile([128, B], f32, tag="ob32")
                            nc.vector.tensor_scalar(
                                out=ob32[:, :nb], in0=psh[:, :nb],
                                scalar1=s_bias[:, 1:2], scalar2=None,
                                op0=mybir.AluOpType.add)
                            nc.sync.dma_start(out=t_out.ap()[:, cols],
                                              in_=ob32[:, :nb])
                            ob16 = ep.tile([128, B], f16, tag="ob16")
                            nc.scalar.activation(
                                out=ob16[:, :nb], in_=ob32[:, :nb],
                                func=mybir.ActivationFunctionType.Copy)
                            for wmat, bcol, tdst, tg in (
                                    (wv, 2, t_xv, "xv"), (wt, 3, t_xt, "xt")):
                                ps3 = psa.tile([128, B], f32, tag="agg",
                                               name="agg")
                                nc.tensor.matmul(ps3[:, :nb], lhsT=wmat[:],
                                                 rhs=ob16[:, :nb],
                                                 start=True, stop=True)
                                vb = ep.tile([128, B], f32, tag=tg)
                                nc.vector.tensor_scalar(
                                    out=vb[:, :nb], in0=ps3[:, :nb],
                                    scalar1=s_bias[:, bcol:bcol + 1],
                                    scalar2=None, op0=mybir.AluOpType.add)
                                nc.scalar.activation(
                                    out=vb[:, :nb], in_=vb[:, :nb],
                                    func=mybir.ActivationFunctionType.Relu)
                                nc.sync.dma_start(out=tdst.ap()[:, cols],
                                                  in_=vb[:, :nb])
                    if l < LAYERS - 1 and g in AG_GROUP:
                        r = AG_GROUP[g]
                        nc.gpsimd.collective_compute(
                            "AllGather", mybir.AluOpType.bypass,
                            replica_groups=RG,
                            ins=[agin_nxt[r].ap()],
                            outs=[tbl_nxt[r].ap()])

    nc.compile()
    return nc


def _install_profile_hook():
    """Dev-only: register the axon NTFF profiling hook (KERNEL_TRACE=1)."""
    import sys
    import types
    if "antenv.axon_hooks" in sys.modules:
        return
    try:
        mod = types.ModuleType("antenv.axon_hooks")
        state = {"hook": None}
        mod.set_axon_ntff_profile_hook = lambda h: state.__setitem__("hook", h)
        mod.get_axon_ntff_profile_hook = lambda: state["hook"]
        sys.modules["antenv.axon_hooks"] = mod
        import antenv
        antenv.axon_hooks = mod
        sys.path.insert(0, "/root/.axon_site")
        from trn_agent_boot.trn_boot import _ntff_profile_via_ctypes
        mod.set_axon_ntff_profile_hook(
            _ntff_profile_via_ctypes("/opt/axon/libaxon_pjrt.so"))
    except Exception as e:  # profiling is best-effort
        print("profile hook install failed:", e)


def kernel(**inputs):
    x = np.asarray(inputs["x"], dtype=np.float32)
    ei = np.asarray(inputs["edge_index"]).astype(np.int64)
    W1 = np.asarray(inputs["W1"], np.float32)
    b1 = np.asarray(inputs["b1"], np.float32)
    conv_w = np.asarray(inputs["conv_w"], np.float32)
    W2 = np.asarray(inputs["W2"], np.float32)
    b2 = np.asarray(inputs["b2"], np.float32)
    Wv = np.asarray(inputs["Wv"], np.float32)
    bv = np.asarray(inputs["bv"], np.float32)
    Wt = np.asarray(inputs["Wt"], np.float32)
    bt = np.asarray(inputs["bt"], np.float32)

    src = ei[0]
    dst = ei[1]

    # ---- node balancing permutation (within each core-quarter slice) ----
    d4 = np.zeros((N, NQ), np.int32)
    np.add.at(d4, (dst, _reg_of(src % SHARD)), 1)
    node_at = _balance(d4)
    inv_perm = np.empty(N, np.int64)
    inv_perm[node_at] = np.arange(N)
    x = x[node_at]
    src = inv_perm[src]
    dst = inv_perm[dst]

    deg = np.bincount(dst, minlength=N).astype(np.float64) + 1.0
    dinv = 1.0 / np.sqrt(deg)
    croot = np.sqrt(1.0 - ALPHA)

    a_src = src
    a_dst = dst

    core = a_dst // SHARD
    blk = (a_dst % SHARD) // B
    s_c = a_src // SHARD
    s_j = a_src % SHARD
    quad = _reg_of(s_j)
    rsz_a = np.array(RSZ)[quad]
    rb_a = np.array(RBASE)[quad]
    g_idx = s_c * rsz_a + (s_j - rb_a)           # idx within table region

    cell = (core * NBLK + blk) * NQ + quad
    ncell = NCORES * NBLK * NQ
    cnt = np.bincount(cell, minlength=ncell).reshape(NCORES, NBLK, NQ)
    v = cnt.max(axis=0)

    calls, total_slots, total_pieces = _build_layout(v)
    maxcall = max(cl["nch"] for cl in calls)
    maxpieces = max(len(cl["pieces"]) for cl in calls)
    nchb_of = np.ones(NBLK, np.int64)          # +1: self-loop matmul
    for cl in calls:
        for (cch, b) in cl["pieces"]:
            nchb_of[b] += 1

    order = np.argsort(cell, kind="stable")
    cs = cell[order]
    counts = np.bincount(cell, minlength=ncell)
    pos0 = np.concatenate([[0], np.cumsum(counts)[:-1]])
    rank = np.arange(len(cs)) - pos0[cs]
    b_o = (cs // NQ) % NBLK
    q_o = cs % NQ
    c_o = cs // (NQ * NBLK)
    cell_start = np.zeros((NBLK, NQ), np.int64)
    for cl in calls:
        blo = cl["g"] * GRP
        for bi, b in enumerate(range(blo, min(blo + GRP, NBLK))):
            cell_start[b, cl["q"]] = cl["scol"] + cl["off"][bi]
    slot = cell_start[b_o, q_o] + rank

    idx_arr = np.zeros((NCORES, total_slots), np.int32)
    dl_arr = np.full((NCORES, total_slots), -1.0, np.float32)
    idx_arr[c_o, slot] = g_idx[order]
    dl_arr[c_o, slot] = (a_dst[order] % SHARD - b_o * B).astype(np.float32)

    # piece dl columns [NCORES, 128, total_pieces], masked to each cell
    dlp = np.full((NCORES, 128, total_pieces), -1.0, np.float32)
    for cl in calls:
        blo = cl["g"] * GRP
        for pi, (cch, b) in enumerate(cl["pieces"]):
            bi = b - blo
            lo, hi = int(cl["off"][bi]), int(cl["off"][bi + 1])
            p0 = cch * 128
            s0, s1 = max(lo, p0), min(hi, p0 + 128)
            lanes = np.arange(s0 - p0, s1 - p0)
            slots_r = cl["scol"] + np.arange(s0, s1)
            dlp[:, lanes, cl["pbase"] + pi] = dl_arr[:, slots_r]
    print(f"kernel_v4: slots={total_slots} pieces={total_pieces} "
          f"calls/layer={len(calls)} maxcall={maxcall} "
          f"maxpieces={maxpieces} descs={sum(c['nidx'] for c in calls)}")

    # weights pack
    wpack = np.zeros((14 * 128, HID), np.float32)
    wpack[0:128] = W1[0:128]
    wpack[128:256] = W1[128:256]
    for l in range(LAYERS):
        beta = float(np.log(THETA / (l + 1) + 1.0))
        wpack[(2 + l) * 128:(3 + l) * 128] = \
            (1.0 - beta) * np.eye(HID, dtype=np.float32) + beta * conv_w[l]
    wpack[10 * 128:11 * 128] = W2
    wpack[11 * 128:12 * 128] = Wv
    wpack[12 * 128:13 * 128] = Wt
    wpack[13 * 128:14 * 128] = np.tile(b1, (128, 1))
    wpack16 = wpack.astype(np.float16)
    biaspk = np.stack([b1, b2, bv, bt], axis=1).astype(np.float32)
    iota16 = np.tile(np.arange(128, dtype=np.float16), (128, 1))

    in_maps = []
    for c in range(NCORES):
        dv = np.zeros((128, NBLK), np.float32)
        rows = np.arange(SHARD) + c * SHARD
        dvflat = (croot * dinv[rows]).astype(np.float32)
        dv[:, :SHARD // B] = dvflat[:(SHARD // B) * B].reshape(-1, B).T
        tail = SHARD - (SHARD // B) * B
        if tail:
            dv[:tail, NBLK - 1] = dvflat[(SHARD // B) * B:]
        wrow = np.tile(dvflat.astype(np.float16), (128, 1))
        in_maps.append({
            "xT": np.ascontiguousarray(
                x[c * SHARD:(c + 1) * SHARD].T).astype(np.float16),
            "gidx": _wrap_idx(idx_arr[c]),
            "dl16": np.ascontiguousarray(dlp[c].astype(np.float16)),
            "wrow": np.ascontiguousarray(wrow),
            "iota": iota16,
            "ident": np.eye(128, dtype=np.float16),
            "dinvrow": dv,
            "wpack": wpack16,
            "bias": biaspk,
        })

    nc = _build_program(calls, total_slots, total_pieces, maxcall, maxpieces,
                        nchb_of)

    trace = os.environ.get("KERNEL_TRACE", "0") == "1"
    if trace:
        _install_profile_hook()
    res = run_bass_kernel_spmd(nc, in_maps, core_ids=list(range(NCORES)),
                               trace=trace)
    if trace:
        kernel.last_res = res

    def unshard(key):
        full = np.concatenate([np.asarray(res.results[c][key]).T
                               for c in range(NCORES)], axis=0)
        outp = np.empty_like(full)
        outp[node_at] = full
        return outp

    return (unshard("out_s"), unshard("xv_s"), unshard("xt_s"))


# revision 28
# speedup vs baseline: 1.1420x; 1.0311x over previous
"""GCNII kernel v4 for 8 Trainium2 NeuronCores.

Per layer: per-edge source rows are fetched with dma_gather (fp16, 256B
descs, 4 queues) from the AllGathered node table; scatter one-hot tiles
are built ON-CHIP on the Vector engine (iota==dl) from resident per-piece
dl columns (layer-invariant); the per-edge weight factorizes as
norm = (c*dinv[src]) * (c*dinv[dst]): the src factor is pre-scaled into
the table rows, the dst factor is a per-column multiply in the epilogue,
so the one-hot is pure 0/1 and self-loops are ordinary slots.

Cells (dst-block x src-quad) are packed UNALIGNED at their max-over-cores
size after a host-side node-balancing permutation (within each quarter),
so descriptor count ~= edge count.  Chunks that straddle cell boundaries
issue one matmul per overlapped block ("pieces") with a masked dl column.
Region-pipelined AllGather (4 sub-shards per layer).
"""
import os
import numpy as np

import concourse.bass as bass
import concourse.bacc as bacc
import concourse.mybir as mybir
import concourse.tile as tile
from concourse.bass_utils import run_bass_kernel_spmd

f32 = mybir.dt.float32
f16 = mybir.dt.float16
i16 = mybir.dt.int16
i32 = mybir.dt.int32
f8 = mybir.dt.float8e4

N = 100000
E = 1000000
IN_DIM = 256
HID = 128
LAYERS = 8
ALPHA = 0.1
THETA = 0.5
NCORES = 8
SHARD = N // NCORES          # 12500
B = 128
NBLK = -(-SHARD // B)        # 98
NQ = 4
GRP = 3
NGRP = -(-NBLK // GRP)       # 33
RSZ = [3200, 3200, 3200, 2900]           # rows per core per region
RBASE = [0, 3200, 6400, 9600]
TRSZ = [r * NCORES for r in RSZ]         # table region rows
AG_GROUP = {}
for _r, _last in ((0, 24), (1, 49), (2, 74), (3, 97)):
    AG_GROUP[_last // GRP] = _r


def _reg_of(j):
    return np.minimum(j // 3200, 3)


def _wrap_idx(a):
    s = a.reshape(-1, 16).T.astype(np.int16)
    return np.tile(s, (8, 1))


def _balance(d4):
    """Mean-targeted balance within each (core, quarter); returns node_at
    (new position -> node id)."""
    node_at = np.empty(N, np.int64)
    for c in range(NCORES):
        for qt in range(4):
            lo = c * SHARD + RBASE[qt]
            n = RSZ[qt]
            nodes = np.arange(lo, lo + n)
            nb = -(-n // B)
            caps = np.full(nb, B, np.int64)
            caps[-1] = n - (nb - 1) * B
            dv = d4[nodes].astype(np.float64)
            capsf = (caps / B)[:, None]
            target = dv.sum(0)[None, :] / nb * capsf
            order_n = np.argsort(-dv.max(1), kind="stable")
            sums = np.zeros((nb, NQ))
            fill = np.zeros(nb, np.int64)
            bins_of = np.zeros(n, np.int64)
            scale = np.sqrt(np.maximum(target, 1.0))
            for i in order_n:
                dev = ((sums + dv[i]) - target) / scale
                score = dev.max(1) + 1e9 * (fill >= caps)
                j = int(np.argmin(score))
                sums[j] += dv[i]
                bins_of[i] = j
                fill[j] += 1
            node_at[lo:lo + n] = nodes[np.argsort(bins_of, kind="stable")]
    return node_at


def _build_layout(v):
    """v[b, q] = cell capacity (max count over cores).

    Returns calls: list of dicts (g, q, scol, nidx, nch, pbase, pieces,
    off), plus total slot columns (chunk-padded) and total piece count.
    """
    calls = []
    scol = 0
    pcol = 0
    for g in range(NGRP):
        blo, bhi = g * GRP, min((g + 1) * GRP, NBLK)
        for q in range(NQ):
            sizes = [int(v[b, q]) for b in range(blo, bhi)]
            off = np.concatenate([[0], np.cumsum(sizes)]).astype(np.int64)
            S = int(off[-1])
            nidx = -(-S // 16) * 16
            nch = -(-nidx // 128) if nidx else 0
            pieces = []
            for cch in range(nch):
                w0, w1 = cch * 128, cch * 128 + 128
                for bi, b in enumerate(range(blo, bhi)):
                    if off[bi] < w1 and off[bi + 1] > w0 and sizes[bi] > 0:
                        pieces.append((cch, b))
            calls.append(dict(g=g, q=q, scol=scol, nidx=nidx, nch=nch,
                              pbase=pcol, pieces=pieces, off=off))
            scol += nch * 128
            pcol += len(pieces)
    return calls, scol, pcol


def _build_program(calls, total_slots, total_pieces, maxcall, maxpieces,
                   nchb_of):
    nc = bacc.Bacc("TRN2", target_bir_lowering=False, debug=False,
                   num_devices=NCORES, num_swdge_queues=NQ)

    t_xT = nc.dram_tensor("xT", [IN_DIM, SHARD], f16, kind="ExternalInput")
    t_idx = nc.dram_tensor("gidx", [128, total_slots // 16], i16,
                           kind="ExternalInput")
    t_dl = nc.dram_tensor("dl16", [128, total_pieces], f16,
                          kind="ExternalInput")
    t_wrow = nc.dram_tensor("wrow", [128, SHARD], f16, kind="ExternalInput")
    t_iota = nc.dram_tensor("iota", [128, 128], f16, kind="ExternalInput")
    t_ident = nc.dram_tensor("ident", [128, 128], f16, kind="ExternalInput")
    t_dinv = nc.dram_tensor("dinvrow", [128, NBLK], f32, kind="ExternalInput")
    NW = 14
    t_wp = nc.dram_tensor("wpack", [NW * 128, HID], f16, kind="ExternalInput")
    t_bias = nc.dram_tensor("bias", [128, 4], f32, kind="ExternalInput")

    t_out = nc.dram_tensor("out_s", [128, SHARD], f32, kind="ExternalOutput")
    t_xv = nc.dram_tensor("xv_s", [128, SHARD], f32, kind="ExternalOutput")
    t_xt = nc.dram_tensor("xt_s", [128, SHARD], f32, kind="ExternalOutput")

    t_tbl = [[nc.dram_tensor(f"tbl{i}_{r}", [TRSZ[r], HID], f16,
                             kind="Internal", addr_space="Shared")
              for r in range(4)] for i in range(2)]
    t_agin = [[nc.dram_tensor(f"agin{i}_{r}", [RSZ[r], HID], f16,
                              kind="Internal") for r in range(4)]
              for i in range(2)]

    RG = [list(range(NCORES))]

    with tile.TileContext(nc) as tc:
        with (
            tc.tile_pool(name="persist", bufs=1) as pp,
            tc.tile_pool(name="gp", bufs=10) as gp,
            tc.tile_pool(name="ohs", bufs=8) as ohs,
            tc.tile_pool(name="ep", bufs=8) as ep,
            tc.tile_pool(name="psa", bufs=6, space="PSUM") as psa,
            tc.tile_pool(name="pse", bufs=2, space="PSUM") as pse,
        ):
            s_idx = pp.tile([128, total_slots // 16], i16)
            nc.sync.dma_start(out=s_idx[:], in_=t_idx.ap())
            s_dl = pp.tile([128, total_pieces], f16)
            nc.sync.dma_start(out=s_dl[:], in_=t_dl.ap())
            s_wrow = pp.tile([128, SHARD], f16)
            nc.sync.dma_start(out=s_wrow[:], in_=t_wrow.ap())
            s_iota = pp.tile([128, 128], f16)
            nc.sync.dma_start(out=s_iota[:], in_=t_iota.ap())
            s_ident = pp.tile([128, 128], f16)
            nc.sync.dma_start(out=s_ident[:], in_=t_ident.ap())
            t_loc = pp.tile([128, NBLK * HID], f16, tag="tloc")
            s_dinv = pp.tile([128, NBLK], f32)
            nc.sync.dma_start(out=s_dinv[:], in_=t_dinv.ap())
            sw = []
            for k in range(NW):
                w = pp.tile([128, HID], f16, tag=f"w{k}")
                nc.sync.dma_start(out=w[:], in_=t_wp.ap()[k * 128:(k + 1) * 128, :])
                sw.append(w)
            (w1a, w1b) = sw[0:2]
            su = sw[2:10]
            w2, wv, wt, b1row = sw[10:14]
            s_bias = pp.tile([128, 4], f32)
            nc.sync.dma_start(out=s_bias[:], in_=t_bias.ap())
            h0T = pp.tile([128, SHARD], f16, tag="h0T")
            h8T = pp.tile([128, SHARD], f16, tag="h8T")

            # zero gather bufs once: trailing non-gathered lanes of partial
            # chunks must hold finite values for the 0-weighted matmul.
            for _ in range(10):
                gz = gp.tile([128, maxcall * 128], f16, tag="gt", name="gt")
                nc.vector.memset(gz[:], 0.0)

            def nb_of(b):
                return min(B, SHARD - b * B)

            # ---------------- W1 phase (feature-major h) ----------------
            for b in range(NBLK):
                nb = nb_of(b)
                cols = slice(b * B, b * B + nb)
                xa = ep.tile([128, B], f16, tag="xa")
                xb = ep.tile([128, B], f16, tag="xb")
                nc.sync.dma_start(out=xa[:, :nb], in_=t_xT.ap()[0:128, cols])
                nc.sync.dma_start(out=xb[:, :nb], in_=t_xT.ap()[128:256, cols])
                ps = psa.tile([128, B], f32, tag="agg", name="agg")
                nc.tensor.matmul(ps[:, :nb], lhsT=w1a[:], rhs=xa[:, :nb],
                                 start=True, stop=False)
                nc.tensor.matmul(ps[:, :nb], lhsT=w1b[:], rhs=xb[:, :nb],
                                 start=False, stop=True)
                hT = ep.tile([128, B], f16, tag="hT")
                nc.vector.tensor_scalar(out=hT[:, :nb], in0=ps[:, :nb],
                                        scalar1=s_bias[:, 0:1], scalar2=None,
                                        op0=mybir.AluOpType.add)
                nc.scalar.activation(out=h0T[:, cols], in_=hT[:, :nb],
                                     func=mybir.ActivationFunctionType.Relu,
                                     scale=ALPHA)
                ps_n = pse.tile([128, B], f32, tag="pp")
                nc.tensor.matmul(ps_n[:nb, :], lhsT=xa[:, :nb], rhs=w1a[:],
                                 start=True, stop=False)
                nc.tensor.matmul(ps_n[:nb, :], lhsT=xb[:, :nb], rhs=w1b[:],
                                 start=False, stop=True)
                hrow = ep.tile([128, HID], f16, tag="hrow")
                nc.vector.tensor_tensor(out=hrow[:nb, :], in0=ps_n[:nb, :],
                                        in1=b1row[:nb, :],
                                        op=mybir.AluOpType.add)
                # table rows pre-scaled: t = relu(h * c*dinv)  (dinv > 0)
                nc.scalar.activation(out=t_loc[:nb, b * HID:b * HID + HID],
                                     in_=hrow[:nb, :],
                                     func=mybir.ActivationFunctionType.Relu,
                                     scale=s_dinv[:nb, b:b + 1])
                r = min(b // 25, 3)
                rows = slice(b * B - RBASE[r], b * B - RBASE[r] + nb)
                nc.sync.dma_start(out=t_agin[0][r].ap()[rows, :],
                                  in_=t_loc[:nb, b * HID:b * HID + HID])
                if b in (24, 49, 74, 97):
                    nc.gpsimd.collective_compute(
                        "AllGather", mybir.AluOpType.bypass,
                        replica_groups=RG,
                        ins=[t_agin[0][b // 25].ap()],
                        outs=[t_tbl[0][b // 25].ap()])

            # ---------------- conv layers ----------------
            for l in range(LAYERS):
                tbl_cur = t_tbl[l % 2]
                tbl_nxt = t_tbl[(l + 1) % 2]
                agin_nxt = t_agin[(l + 1) % 2]
                for g in range(NGRP):
                    blo, bhi = g * GRP, min((g + 1) * GRP, NBLK)
                    aggs = [psa.tile([128, B], f32, tag="agg", name="agg")
                            for _ in range(bhi - blo)]
                    gcalls = [cl for cl in calls
                              if cl["g"] == g and cl["nch"] > 0]
                    gts = {}
                    for cl in gcalls:
                        q = cl["q"]
                        gt = gp.tile([128, maxcall * 128], f16, tag="gt",
                                     name="gt")
                        nc.gpsimd.dma_gather(
                            out_ap=gt[:, :cl["nch"] * 128].rearrange(
                                "p (c f) -> p c f", f=HID),
                            in_ap=tbl_cur[q].ap(),
                            idxs_ap=s_idx[:, cl["scol"] // 16:
                                          cl["scol"] // 16 + cl["nidx"] // 16],
                            num_idxs=cl["nidx"],
                            num_idxs_reg=cl["nidx"],
                            elem_size=HID,
                            single_packet=False,
                            queue_num=q,
                        )
                        npc = len(cl["pieces"])
                        oh = ohs.tile([128, maxpieces * 128], f8, tag="oh",
                                      name="oh")
                        nc.vector.tensor_tensor(
                            out=oh[:, :npc * 128].rearrange(
                                "p (k f) -> p k f", f=128),
                            in0=s_iota[:].unsqueeze(1).broadcast_to(
                                [128, npc, 128]),
                            in1=s_dl[:, cl["pbase"]:cl["pbase"] + npc
                                     ].unsqueeze(2).broadcast_to(
                                [128, npc, 128]),
                            op=mybir.AluOpType.is_equal)
                        gts[q] = (gt, oh, cl)
                    done = {}
                    for b in range(blo, bhi):
                        nb = nb_of(b)
                        nc.tensor.matmul(
                            aggs[b - blo][:, :nb],
                            lhsT=t_loc[:nb, b * HID:b * HID + HID],
                            rhs=s_ident[:nb, :nb],
                            start=True, stop=(nchb_of[b] == 1))
                        done[b] = 1
                    for q in range(NQ):
                        if q not in gts:
                            continue
                        gt, oh, cl = gts[q]
                        for pi, (cch, b) in enumerate(cl["pieces"]):
                            ps_t = aggs[b - blo]
                            k = done[b]
                            nc.tensor.matmul(
                                ps_t[:],
                                lhsT=gt[:, cch * 128:(cch + 1) * 128],
                                rhs=oh[:, pi * 128:(pi + 1) * 128],
                                start=(k == 0), stop=(k == nchb_of[b] - 1))
                            done[b] = k + 1
                    # epilogue (feature-major agg)
                    for b in range(blo, bhi):
                        nb = nb_of(b)
                        cols = slice(b * B, b * B + nb)
                        ps_t = aggs[b - blo]
                        t0 = ep.tile([128, B], f16, tag="t0")
                        nc.vector.tensor_tensor(
                            out=t0[:, :nb], in0=ps_t[:, :nb],
                            in1=s_wrow[:, cols],
                            op=mybir.AluOpType.mult)
                        aggT = ep.tile([128, B], f16, tag="aggT")
                        nc.vector.tensor_tensor(
                            out=aggT[:, :nb], in0=t0[:, :nb],
                            in1=h0T[:, cols],
                            op=mybir.AluOpType.add)
                        if l < LAYERS - 1:
                            ps2 = pse.tile([128, B], f32, tag="pp")
                            nc.tensor.matmul(ps2[:nb, :], lhsT=aggT[:, :nb],
                                             rhs=su[l][:], start=True,
                                             stop=True)
                            nc.scalar.activation(
                                out=t_loc[:nb, b * HID:b * HID + HID],
                                in_=ps2[:nb, :],
                                func=mybir.ActivationFunctionType.Relu,
                                scale=s_dinv[:nb, b:b + 1])
                            r = min(b // 25, 3)
                            rows = slice(b * B - RBASE[r],
                                         b * B - RBASE[r] + nb)
                            nc.sync.dma_start(out=agin_nxt[r].ap()[rows, :],
                                              in_=t_loc[:nb,
                                                        b * HID:b * HID + HID])
                        else:
                            ps2 = pse.tile([128, B], f32, tag="pp")
                            nc.tensor.matmul(ps2[:, :nb], lhsT=su[l][:],
                                             rhs=aggT[:, :nb],
                                             start=True, stop=True)
                            nc.scalar.activation(
                                out=h8T[:, cols], in_=ps2[:, :nb],
                                func=mybir.ActivationFunctionType.Relu)
                            psh = psa.tile([128, B], f32, tag="agg",
                                           name="agg")
                            nc.tensor.matmul(psh[:, :nb], lhsT=w2[:],
                                             rhs=h8T[:, cols],
                                             start=True, stop=True)
                            ob32 = ep.tile([128, B], f32, tag="ob32")
                            nc.vector.tensor_scalar(
                                out=ob32[:, :nb], in0=psh[:, :nb],
                                scalar1=s_bias[:, 1:2], scalar2=None,
                                op0=mybir.AluOpType.add)
                            nc.sync.dma_start(out=t_out.ap()[:, cols],
                                              in_=ob32[:, :nb])
                            ob16 = ep.tile([128, B], f16, tag="ob16")
                            nc.scalar.activation(
                                out=ob16[:, :nb], in_=ob32[:, :nb],
                                func=mybir.ActivationFunctionType.Copy)
                            for wmat, bcol, tdst, tg in (
                                    (wv, 2, t_xv, "xv"), (wt, 3, t_xt, "xt")):
                                ps3 = psa.tile([128, B], f32, tag="agg",
                                               name="agg")
                                nc.tensor.matmul(ps3[:, :nb], lhsT=wmat[:],
                                                 rhs=ob16[:, :nb],
                                                 start=True, stop=True)
                                vb = ep.tile([128, B], f32, tag=tg)
                                nc.vector.tensor_scalar(
                                    out=vb[:, :nb], in0=ps3[:, :nb],
                                    scalar1=s_bias[:, bcol:bcol + 1],
                                    scalar2=None, op0=mybir.AluOpType.add)
                                nc.scalar.activation(
                                    out=vb[:, :nb], in_=vb[:, :nb],
                                    func=mybir.ActivationFunctionType.Relu)
                                nc.sync.dma_start(out=tdst.ap()[:, cols],
                                                  in_=vb[:, :nb])
                    if l < LAYERS - 1 and g in AG_GROUP:
                        r = AG_GROUP[g]
                        nc.gpsimd.collective_compute(
                            "AllGather", mybir.AluOpType.bypass,
                            replica_groups=RG,
                            ins=[agin_nxt[r].ap()],
                            outs=[tbl_nxt[r].ap()])

    nc.compile()
    return nc


def _install_profile_hook():
    """Dev-only: register the axon NTFF profiling hook (KERNEL_TRACE=1)."""
    import sys
    import types
    if "antenv.axon_hooks" in sys.modules:
        return
    try:
        mod = types.ModuleType("antenv.axon_hooks")
        state = {"hook": None}
        mod.set_axon_ntff_profile_hook = lambda h: state.__setitem__("hook", h)
        mod.get_axon_ntff_profile_hook = lambda: state["hook"]
        sys.modules["antenv.axon_hooks"] = mod
        import antenv
        antenv.axon_hooks = mod
        sys.path.insert(0, "/root/.axon_site")
        from trn_agent_boot.trn_boot import _ntff_profile_via_ctypes
        mod.set_axon_ntff_profile_hook(
            _ntff_profile_via_ctypes("/opt/axon/libaxon_pjrt.so"))
    except Exception as e:  # profiling is best-effort
        print("profile hook install failed:", e)


def kernel(**inputs):
    x = np.asarray(inputs["x"], dtype=np.float32)
    ei = np.asarray(inputs["edge_index"]).astype(np.int64)
    W1 = np.asarray(inputs["W1"], np.float32)
    b1 = np.asarray(inputs["b1"], np.float32)
    conv_w = np.asarray(inputs["conv_w"], np.float32)
    W2 = np.asarray(inputs["W2"], np.float32)
    b2 = np.asarray(inputs["b2"], np.float32)
    Wv = np.asarray(inputs["Wv"], np.float32)
    bv = np.asarray(inputs["bv"], np.float32)
    Wt = np.asarray(inputs["Wt"], np.float32)
    bt = np.asarray(inputs["bt"], np.float32)

    src = ei[0]
    dst = ei[1]

    # ---- node balancing permutation (within each core-quarter slice) ----
    d4 = np.zeros((N, NQ), np.int32)
    np.add.at(d4, (dst, _reg_of(src % SHARD)), 1)
    node_at = _balance(d4)
    inv_perm = np.empty(N, np.int64)
    inv_perm[node_at] = np.arange(N)
    x = x[node_at]
    src = inv_perm[src]
    dst = inv_perm[dst]

    deg = np.bincount(dst, minlength=N).astype(np.float64) + 1.0
    dinv = 1.0 / np.sqrt(deg)
    croot = np.sqrt(1.0 - ALPHA)

    a_src = src
    a_dst = dst

    core = a_dst // SHARD
    blk = (a_dst % SHARD) // B
    s_c = a_src // SHARD
    s_j = a_src % SHARD
    quad = _reg_of(s_j)
    rsz_a = np.array(RSZ)[quad]
    rb_a = np.array(RBASE)[quad]
    g_idx = s_c * rsz_a + (s_j - rb_a)           # idx within table region

    cell = (core * NBLK + blk) * NQ + quad
    ncell = NCORES * NBLK * NQ
    cnt = np.bincount(cell, minlength=ncell).reshape(NCORES, NBLK, NQ)
    v = cnt.max(axis=0)

    calls, total_slots, total_pieces = _build_layout(v)
    maxcall = max(cl["nch"] for cl in calls)
    maxpieces = max(len(cl["pieces"]) for cl in calls)
    nchb_of = np.ones(NBLK, np.int64)          # +1: self-loop matmul
    for cl in calls:
        for (cch, b) in cl["pieces"]:
            nchb_of[b] += 1

    order = np.argsort(cell, kind="stable")
    cs = cell[order]
    counts = np.bincount(cell, minlength=ncell)
    pos0 = np.concatenate([[0], np.cumsum(counts)[:-1]])
    rank = np.arange(len(cs)) - pos0[cs]
    b_o = (cs // NQ) % NBLK
    q_o = cs % NQ
    c_o = cs // (NQ * NBLK)
    cell_start = np.zeros((NBLK, NQ), np.int64)
    for cl in calls:
        blo = cl["g"] * GRP
        for bi, b in enumerate(range(blo, min(blo + GRP, NBLK))):
            cell_start[b, cl["q"]] = cl["scol"] + cl["off"][bi]
    slot = cell_start[b_o, q_o] + rank

    idx_arr = np.zeros((NCORES, total_slots), np.int32)
    dl_arr = np.full((NCORES, total_slots), -1.0, np.float32)
    idx_arr[c_o, slot] = g_idx[order]
    dl_arr[c_o, slot] = (a_dst[order] % SHARD - b_o * B).astype(np.float32)

    # piece dl columns [NCORES, 128, total_pieces], masked to each cell
    dlp = np.full((NCORES, 128, total_pieces), -1.0, np.float32)
    for cl in calls:
        blo = cl["g"] * GRP
        for pi, (cch, b) in enumerate(cl["pieces"]):
            bi = b - blo
            lo, hi = int(cl["off"][bi]), int(cl["off"][bi + 1])
            p0 = cch * 128
            s0, s1 = max(lo, p0), min(hi, p0 + 128)
            lanes = np.arange(s0 - p0, s1 - p0)
            slots_r = cl["scol"] + np.arange(s0, s1)
            dlp[:, lanes, cl["pbase"] + pi] = dl_arr[:, slots_r]
    print(f"kernel_v4: slots={total_slots} pieces={total_pieces} "
          f"calls/layer={len(calls)} maxcall={maxcall} "
          f"maxpieces={maxpieces} descs={sum(c['nidx'] for c in calls)}")

    # weights pack
    wpack = np.zeros((14 * 128, HID), np.float32)
    wpack[0:128] = W1[0:128]
    wpack[128:256] = W1[128:256]
    for l in range(LAYERS):
        beta = float(np.log(THETA / (l + 1) + 1.0))
        wpack[(2 + l) * 128:(3 + l) * 128] = \
            (1.0 - beta) * np.eye(HID, dtype=np.float32) + beta * conv_w[l]
    wpack[10 * 128:11 * 128] = W2
    wpack[11 * 128:12 * 128] = Wv
    wpack[12 * 128:13 * 128] = Wt
    wpack[13 * 128:14 * 128] = np.tile(b1, (128, 1))
    wpack16 = wpack.astype(np.float16)
    biaspk = np.stack([b1, b2, bv, bt], axis=1).astype(np.float32)
    iota16 = np.tile(np.arange(128, dtype=np.float16), (128, 1))

    in_maps = []
    for c in range(NCORES):
        dv = np.zeros((128, NBLK), np.float32)
        rows = np.arange(SHARD) + c * SHARD
        dvflat = (croot * dinv[rows]).astype(np.float32)
        dv[:, :SHARD // B] = dvflat[:(SHARD // B) * B].reshape(-1, B).T
        tail = SHARD - (SHARD // B) * B
        if tail:
            dv[:tail, NBLK - 1] = dvflat[(SHARD // B) * B:]
        wrow = np.tile(dvflat.astype(np.float16), (128, 1))
        in_maps.append({
            "xT": np.ascontiguousarray(
                x[c * SHARD:(c + 1) * SHARD].T).astype(np.float16),
            "gidx": _wrap_idx(idx_arr[c]),
            "dl16": np.ascontiguousarray(dlp[c].astype(np.float16)),
            "wrow": np.ascontiguousarray(wrow),
            "iota": iota16,
            "ident": np.eye(128, dtype=np.float16),
            "dinvrow": dv,
            "wpack": wpack16,
            "bias": biaspk,
        })

    nc = _build_program(calls, total_slots, total_pieces, maxcall, maxpieces,
                        nchb_of)

    trace = os.environ.get("KERNEL_TRACE", "0") == "1"
    if trace:
        _install_profile_hook()
    res = run_bass_kernel_spmd(nc, in_maps, core_ids=list(range(NCORES)),
                               trace=trace)
    if trace:
        kernel.last_res = res

    def unshard(key):
        full = np.concatenate([np.asarray(res.results[c][key]).T
                               for c in range(NCORES)], axis=0)
        outp = np.empty_like(full)
        outp[node_at] = full
        return outp

    return (unshard("out_s"), unshard("xv_s"), unshard("xt_s"))
